# revision 14
# baseline (speedup 1.0000x reference)
"""DepthConsistencyLoss Trainium2 kernel (8 NeuronCores, batch-parallel).

loss = mean_{n,l} sum_{r=0..188} w_{r%9}[l] * (cam_unfold[r,l] - cam_center[r%21,l])^2

Restructured (verified exactly against the reference):
  loss*N*H*W = sum_n ( term1 - 2*term2 + term3 ) with, per batch element n:
    term1 = sum_p sum_l w_p * S_{dp}(E)        E = sum_c cam_c^2
    term2 = sum_g sum_{p in g} sum_l w_p * S_{dp}(Pi_g)
            Pi_g = sum_{c0} P_c0 * S_{(dy,0)}(cam_{c'})   (13 distinct products)
            P_c0 = cam_c0 + cam_{c0+7} + cam_{c0+14}
    term3 = 3 * sum_{c'} sum_l G_c' * Omega_c'            G = cam^2
            Omega from 9 shifted partial sums of wsum_m = w_m+w_{m+3}+w_{m+6}
  w_p = wspat_p * exp(-50*(S_{dp}(D) - D)^2), w_4 == 1.

Layout: partitions = 112 image rows per y-tile (2 tiles), free dim = [img][x]
(x padded 2+2 -> 228). Compute APs always start at partition 0 (HW rule:
start must be 0/32/64/96). All y-shifted operands are DMA-materialized
version buffers (partition-offset DMA is legal), with cross-tile slivers
for rows crossing the tile boundary and zero padding at image edges.
x-shifts are free-dim offsets.

Each core handles one batch element; host sums the 8 x [2,112,24] partials.
"""

import os
import sys

import numpy as np

for _p in ("/opt/trn_rl_repo", os.path.expanduser("~/.axon_site/_ro/trn_rl_repo")):
    if os.path.isdir(_p) and _p not in sys.path:
        sys.path.insert(0, _p)

import concourse.bass as bass
import concourse.bacc as bacc
import concourse.tile as tile
from concourse import mybir
from concourse.bass_utils import run_bass_kernel_spmd

F32 = mybir.dt.float32
BF16 = mybir.dt.bfloat16
Alu = mybir.AluOpType
Act = mybir.ActivationFunctionType

N, C, H, W = 8, 21, 224, 224
XF = 228
X0, X1 = 2, 226
NP = 112           # partitions per y-tile = core rows
NACC = 24
SIGMA_S = 5.0
STAGE = int(os.environ.get("DCL_STAGE", "4"))


def _delta(p):
    return (p // 3 - 1, p % 3 - 1)


def _cp_of_j(j):
    row = 84 + j
    return row // 9, row % 9


def _tables():
    table = {}
    for g in range(3):
        for c0 in range(7):
            ents = []
            for p in (3 * g, 3 * g + 1, 3 * g + 2):
                j = (9 * c0 + p) % 21
                cpr, ppr = _cp_of_j(j)
                dpy, dpx = _delta(p)
                dqy, dqx = _delta(ppr)
                ents.append((cpr, dqy - dpy, dqx - dpx))
            assert ents[0] == ents[1] == ents[2], (g, c0, ents)
            cpr, dy, dx = ents[0]
            assert dx == 0
            table[(g, c0)] = (cpr, dy)
    prods = sorted({(dy, c0, cpr) for (g, c0), (cpr, dy) in table.items()})
    pidx = {pr: i for i, pr in enumerate(prods)}
    groups = {
        g: [pidx[(table[(g, c0)][1], c0, table[(g, c0)][0])] for c0 in range(7)]
        for g in range(3)
    }
    return table, prods, groups


def _prod_runs(prods):
    runs = []
    for i, (dy, c0, cpr) in enumerate(prods):
        if runs and runs[-1][0] == dy and c0 == runs[-1][1] + runs[-1][3] \
                and cpr == runs[-1][2] + runs[-1][3]:
            runs[-1][3] += 1
        else:
            runs.append([dy, c0, cpr, 1, i])
    return runs


def _wspat():
    d2 = np.array([(p // 3 - 1) ** 2 + (p % 3 - 1) ** 2 for p in range(9)],
                  dtype=np.float64)
    return np.exp(-d2 / (2.0 * SIGMA_S ** 2))


class _TileCtx:
    """Per-y-tile buffer set."""

    def __init__(self, pool, t):
        self.t = t
        self.stg = pool.tile([NP, C, XF], F32, name=f"stg{t}", tag=f"stg{t}")
        self.dstg = pool.tile([NP, XF], F32, name=f"dstg{t}", tag=f"dstg{t}")
        self.ds = {d: pool.tile([NP, XF], F32, name=f"ds{d}_{t}", tag=f"ds{d}_{t}") for d in (-1, 1)}
        self.camb = pool.tile([NP, 3, XF], BF16, name=f"camb{t}", tag=f"camb{t}")
        self.cs = {d: pool.tile([NP, 3, XF], BF16, name=f"cs{d}_{t}", tag=f"cs{d}_{t}")
                   for d in (-2, -1, 1, 2)}
        self.gsq = pool.tile([NP, C, XF], BF16, name=f"gsq{t}", tag=f"gsq{t}")
        self.etr = pool.tile([NP, 20, XF], BF16, name=f"etr{t}", tag=f"etr{t}")
        self.eimg = pool.tile([NP, XF], BF16, name=f"eimg{t}", tag=f"eimg{t}")
        self.es = {d: pool.tile([NP, XF], BF16, name=f"es{d}_{t}", tag=f"es{d}_{t}") for d in (-1, 1)}
        self.Pb = pool.tile([NP, 7, XF], BF16, name=f"P{t}", tag=f"P{t}")
        self.pdis = pool.tile([NP, 13, XF], BF16, name=f"pdis{t}", tag=f"pdis{t}")
        self.Pi = pool.tile([NP, 3, XF], BF16, name=f"Pi{t}", tag=f"Pi{t}")
        self.pis = {d: pool.tile([NP, 3, XF], BF16, name=f"pis{d}_{t}", tag=f"pis{d}_{t}")
                    for d in (-1, 1)}
        self.wb = pool.tile([NP, 9, XF], BF16, name=f"w{t}", tag=f"w{t}")
        self.wsb = pool.tile([NP, 3, XF], BF16, name=f"ws{t}", tag=f"ws{t}")
        self.wss = {d: pool.tile([NP, 3, XF], BF16, name=f"wss{d}_{t}", tag=f"wss{d}_{t}")
                    for d in (-1, 1)}
        self.om = pool.tile([NP, 3, XF], BF16, name=f"om{t}", tag=f"om{t}")
        self.omt = pool.tile([NP, 3, XF], BF16, name=f"omt{t}", tag=f"omt{t}")
        self.ddif = pool.tile([NP, 8, XF], F32, name=f"ddif{t}", tag=f"ddif{t}")
        self.dsq = pool.tile([NP, 8, XF], F32, name=f"dsq{t}", tag=f"dsq{t}")
        self.scr = pool.tile([NP, 3, XF], BF16, name=f"scr{t}", tag=f"scr{t}")
        self.acc = pool.tile([NP, NACC], F32, name=f"acc{t}", tag=f"acc{t}")
        self.bias2 = pool.tile([NP, 2], F32, name=f"bias{t}", tag=f"bias{t}")


def _emit_shift(nc, tcs, t, dst, src_name, dy, nimg):
    """dst[p, ...] = global_src[112*t + p + dy, ...] with zero pad at image edges.

    src_name: attribute on _TileCtx holding the base image buffer (same shape
    as dst). dst must be pre-zeroed. Emits 1-2 DMAs (own part + neighbor sliver).
    """
    def src_of(tt):
        return getattr(tcs[tt], src_name)

    def sl(buf, p0, p1):
        return buf[p0:p1] if nimg == 1 else buf[p0:p1, :, :]

    # own-tile part: rows p with p+dy in [0, NP)
    p0, p1 = max(0, -dy), min(NP, NP - dy)
    nc.sync.dma_start(out=sl(dst, p0, p1), in_=sl(src_of(t), p0 + dy, p1 + dy))
    # neighbor sliver
    if dy > 0 and t == 0:       # rows [NP-dy, NP) come from tile1 rows [0, dy)
        nc.sync.dma_start(out=sl(dst, NP - dy, NP), in_=sl(src_of(1), 0, dy))
    if dy < 0 and t == 1:       # rows [0, -dy) come from tile0 rows [NP+dy, NP)
        nc.sync.dma_start(out=sl(dst, 0, -dy), in_=sl(src_of(0), NP + dy, NP))
    # image-edge rows stay zero (dst pre-memset)


def _emit_tile_pre(nc, tcs, t, cam, dep):
    """Stage 1: loads, conversions, squares, base images (no cross-tile deps)."""
    b = tcs[t]
    v = nc.vector
    s = nc.scalar
    wspat = _wspat()
    y0 = NP * t

    # DMA loads (per-channel; bacc's generate_event_semaphores handles the
    # consumer-side wait fan-in)
    for c in range(C):
        nc.sync.dma_start(out=b.stg[:, c, X0:X1], in_=cam[c, y0:y0 + NP, :])
    v.memset(b.dstg[:, :], 0.0)
    nc.sync.dma_start(out=b.dstg[:, X0:X1], in_=dep[0, y0:y0 + NP, :])

    # zero-init
    v.memset(b.acc[:, :], 0.0)
    v.memset(b.camb[:, :, :], 0.0)
    v.memset(b.Pi[:, :, :], 0.0)
    v.memset(b.wsb[:, :, :], 0.0)
    v.memset(b.wb[:, :, :], 0.0)
    v.memset(b.bias2[:, 0:1], float(np.log(wspat[0])))
    v.memset(b.bias2[:, 1:2], float(np.log(wspat[1])))

    # bf16 C channels
    v.tensor_copy(b.camb[:, :, X0:X1], b.stg[:, 9:12, X0:X1])

    # squares (ACT), f32 in -> bf16 out
    s.activation(out=b.gsq[:, :, X0:X1], in_=b.stg[:, :, X0:X1], func=Act.Square)

    # E tree
    E = 19
    v.tensor_tensor(out=b.etr[:, 0:10, X0:X1], in0=b.gsq[:, 0:20:2, X0:X1],
                    in1=b.gsq[:, 1:20:2, X0:X1], op=Alu.add)
    v.tensor_tensor(out=b.etr[:, 10:15, X0:X1], in0=b.etr[:, 0:10:2, X0:X1],
                    in1=b.etr[:, 1:10:2, X0:X1], op=Alu.add)
    v.tensor_tensor(out=b.etr[:, 15:17, X0:X1], in0=b.etr[:, 10:14:2, X0:X1],
                    in1=b.etr[:, 11:14:2, X0:X1], op=Alu.add)
    v.tensor_tensor(out=b.etr[:, 17, X0:X1], in0=b.etr[:, 15, X0:X1],
                    in1=b.etr[:, 16, X0:X1], op=Alu.add)
    v.tensor_tensor(out=b.etr[:, 18, X0:X1], in0=b.etr[:, 17, X0:X1],
                    in1=b.etr[:, 14, X0:X1], op=Alu.add)
    v.memset(b.eimg[:, :], 0.0)
    v.tensor_tensor(out=b.eimg[:, X0:X1], in0=b.etr[:, 18, X0:X1],
                    in1=b.gsq[:, 20, X0:X1], op=Alu.add)

    # P
    v.tensor_tensor(out=b.Pb[:, :, X0:X1], in0=b.stg[:, 0:7, X0:X1],
                    in1=b.stg[:, 7:14, X0:X1], op=Alu.add)
    v.tensor_tensor(out=b.Pb[:, :, X0:X1], in0=b.Pb[:, :, X0:X1],
                    in1=b.stg[:, 14:21, X0:X1], op=Alu.add)


def _emit_tile_main(nc, tcs, t, out):
    """Stage 2: shifted versions, products, weights, reductions."""
    if STAGE < 2:
        return
    b = tcs[t]
    v = nc.vector
    s = nc.scalar
    table, prods, groups = _tables()

    # shifted C versions (pure-y shifts for the 13 products)
    for d in (-2, -1, 1, 2):
        v.memset(b.cs[d][:, :, :], 0.0)
        _emit_shift(nc, tcs, t, b.cs[d], "camb", d, 3)

    # products, batched by dy (in1 = camb for dy=0 else cs[dy])
    for dy, c00, cp0, n, i0 in _prod_runs(prods):
        src = b.camb if dy == 0 else b.cs[dy]
        v.tensor_tensor(out=b.pdis[:, i0:i0 + n, X0:X1],
                        in0=b.Pb[:, c00:c00 + n, X0:X1],
                        in1=src[:, cp0 - 9:cp0 - 9 + n, X0:X1], op=Alu.mult)

    # Pi_g
    for g in range(3):
        sl = groups[g]
        v.tensor_tensor(out=b.Pi[:, g, X0:X1], in0=b.pdis[:, sl[0], X0:X1],
                        in1=b.pdis[:, sl[1], X0:X1], op=Alu.add)
        for k in sl[2:]:
            v.tensor_tensor(out=b.Pi[:, g, X0:X1], in0=b.Pi[:, g, X0:X1],
                            in1=b.pdis[:, k, X0:X1], op=Alu.add)

    # depth weights
    if STAGE < 3:
        return
    for d in (-1, 1):
        v.memset(b.ds[d][:, :], 0.0)
        _emit_shift(nc, tcs, t, b.ds[d], "dstg", d, 1)
    dmap = [0, 1, 2, 3, 5, 6, 7, 8]
    for i, p in enumerate(dmap):
        dy, dx = _delta(p)
        src = b.dstg if dy == 0 else b.ds[dy]
        v.tensor_tensor(out=b.ddif[:, i, X0:X1],
                        in0=src[:, X0 + dx:X1 + dx],
                        in1=b.dstg[:, X0:X1], op=Alu.subtract)
    s.activation(out=b.dsq[:, :, X0:X1], in_=b.ddif[:, :, X0:X1], func=Act.Square)
    for i, p in enumerate(dmap):
        corner = (p % 3 != 1) and (p // 3 != 1)
        s.activation(out=b.wb[:, p, X0:X1], in_=b.dsq[:, i, X0:X1],
                     func=Act.Exp, scale=-50.0,
                     bias=b.bias2[:, 0:1] if corner else b.bias2[:, 1:2])
    v.memset(b.wb[:, 4, X0:X1], 1.0)

    # wsum
    v.tensor_tensor(out=b.wsb[:, :, X0:X1], in0=b.wb[:, 0:3, X0:X1],
                    in1=b.wb[:, 3:6, X0:X1], op=Alu.add)
    v.tensor_tensor(out=b.wsb[:, :, X0:X1], in0=b.wsb[:, :, X0:X1],
                    in1=b.wb[:, 6:9, X0:X1], op=Alu.add)


def _emit_tile_post(nc, tcs, t, out):
    """Stage 3: cross-tile shifted versions of derived images + reductions."""
    b = tcs[t]
    v = nc.vector
    if STAGE < 4:
        nc.sync.dma_start(out=out[t], in_=b.acc[:, :])
        return

    for d in (-1, 1):
        v.memset(b.es[d][:, :], 0.0)
        _emit_shift(nc, tcs, t, b.es[d], "eimg", d, 1)
        v.memset(b.pis[d][:, :, :], 0.0)
        _emit_shift(nc, tcs, t, b.pis[d], "Pi", d, 3)
        v.memset(b.wss[d][:, :, :], 0.0)
        _emit_shift(nc, tcs, t, b.wss[d], "wsb", d, 3)

    # term1 + term2
    for p in range(9):
        dy, dx = _delta(p)
        g = p // 3
        e_src = b.eimg if dy == 0 else b.es[dy]
        v.affine_mul_reduce(
            out=b.scr[:, p % 3, X0:X1],
            accum_out=b.acc[:, p:p + 1],
            in0=b.wb[:, p, X0:X1],
            in1=e_src[:, X0 + dx:X1 + dx],
            scale=1.0, bias=0.0)
        pi_src = b.Pi if dy == 0 else b.pis[dy]
        v.affine_mul_reduce(
            out=b.scr[:, p % 3, X0:X1],
            accum_out=b.acc[:, 9 + p:10 + p],
            in0=b.wb[:, p, X0:X1],
            in1=pi_src[:, g, X0 + dx:X1 + dx],
            scale=-2.0, bias=0.0)

    # term3
    def _T(q):
        dy, dx = _delta(q)
        src = b.wsb if dy == 0 else b.wss[-dy]
        return src[:, q % 3, X0 - dx:X1 - dx]

    for blk in range(3):
        v.tensor_tensor(out=b.omt[:, blk, X0:X1], in0=_T(3 * blk),
                        in1=_T(3 * blk + 1), op=Alu.add)
        v.tensor_tensor(out=b.omt[:, blk, X0:X1], in0=b.omt[:, blk, X0:X1],
                        in1=_T(3 * blk + 2), op=Alu.add)
    v.tensor_tensor(out=b.om[:, 0, X0:X1], in0=b.omt[:, 1, X0:X1],
                    in1=b.omt[:, 2, X0:X1], op=Alu.add)
    v.tensor_tensor(out=b.om[:, 1, X0:X1], in0=b.om[:, 0, X0:X1],
                    in1=b.omt[:, 0, X0:X1], op=Alu.add)
    v.tensor_tensor(out=b.om[:, 2, X0:X1], in0=b.omt[:, 0, X0:X1],
                    in1=b.omt[:, 1, X0:X1], op=Alu.add)
    v.affine_mul_reduce(
        out=b.scr[:, :, X0:X1],
        accum_out=b.acc[:, 18:19],
        in0=b.gsq[:, 9:12, X0:X1],
        in1=b.om[:, :, X0:X1],
        scale=3.0, bias=0.0)

    nc.sync.dma_start(out=out[t], in_=b.acc[:, :])


def build_nc():
    nc = bacc.Bacc("TRN2", target_bir_lowering=False)
    cam = nc.dram_tensor("cam", (C, H, W), F32, kind="ExternalInput")
    dep = nc.dram_tensor("dep", (1, H, W), F32, kind="ExternalInput")
    out = nc.dram_tensor("out", (2, NP, NACC), F32, kind="ExternalOutput")
    with tile.TileContext(nc) as tc:
        with tc.tile_pool(name="main", bufs=1) as pool:
            tcs = {t: _TileCtx(pool, t) for t in (0, 1)}
            for t in (0, 1):
                _emit_tile_pre(nc, tcs, t, cam, dep)
            for t in (0, 1):
                _emit_tile_main(nc, tcs, t, out)
            for t in (0, 1):
                _emit_tile_post(nc, tcs, t, out)
    nc.finalize()
    return nc


_CACHE = {}


def _get_nc():
    if "nc" not in _CACHE:
        _CACHE["nc"] = build_nc()
    return _CACHE["nc"]


def _run(in_maps, **kw):
    return run_bass_kernel_spmd(_get_nc(), in_maps, core_ids=list(range(N)), **kw)


def _make_in_maps(cam_map, depth_map):
    cam_map = np.ascontiguousarray(cam_map, dtype=np.float32)
    depth_map = np.ascontiguousarray(depth_map, dtype=np.float32)
    return [{"cam": cam_map[i], "dep": depth_map[i]} for i in range(N)]


def kernel(cam_map, depth_map):
    r = _run(_make_in_maps(cam_map, depth_map))
    tot = sum(float(m["out"].astype(np.float64).sum()) for m in r.results)
    return np.array(tot / (N * H * W), dtype=np.float32)
